# revision 3
# baseline (speedup 1.0000x reference)
"""Trainium2 Bass kernel for nn_CMLITargetLoss (v2: fp8 + patch compaction).

Reference semantics (B=64, L=197, D=768):
    sim[b,i,t,p] = text[b,t,:] . image[i,p,:]      (masked where padding_masks[b,p])
    token2patch  = argmax over p of sim[:, :, 1:, 1:]
    only the diagonal (b == i) of token2patch is used:
        aligned[b,t] = image[b, 1 + token2patch[b,b,t]]
        kd_token = mean((text[:,1:] - aligned)^2)
    kd_cls  = mean((image[:,0] - target[:,0])^2)
    loss = kd_cls + kd_token

Algebraic reduction (per sample b, tokens t, patches p):
    S[t,p] = text_t . image_p ; M[t] = max_p S ; O = (S == M) one-hot
    sum_t ||text_t - aligned_t||^2
        = sum ||text||^2 - 2*sum M + sum_p cnt[p]*||image_p||^2

v2 design (vs v1): inputs shipped as fp8e4 (halves HBM traffic; error
contribution ~3e-4); per-sample the ~98 unmasked patches are compacted
host-side into P=128 slots (zero pad) so the whole mask machinery
disappears -- zero-pad columns give S=0 which can never win the argmax
(row maxima are ~+70), and their cnt pairs with ||image_p||^2=0 anyway.
All 8 samples' S matrices live in ONE 4-bank PSUM tile [128,8,2,128] so
max / one-hot / counts run as few big engine ops instead of many tiny
ones (DVE/ACT per-instruction overhead is 58-352 cycles).  ||image_p||^2
comes from the diagonal of the image gram matrix (PE, fp8 squares are
exact in the e6m3 path), extracted with an identity-mask multiply; cnt
is patch-on-partition via matmul(lhsT=O, rhs=ones) at FD=1; the final
dot and all scalar folds happen in one [1,26] PSUM row, combined on
host.  Tokens 68..127 of each chunk-1 slab are zero-filled by a K=1
matmul so reduces never see uninitialised PSUM.
"""

import os
import sys

import numpy as np

for _p in ("/opt/trn_rl_repo", "/root/.axon_site/_ro/trn_rl_repo"):
    if _p not in sys.path and os.path.isdir(_p):
        sys.path.insert(0, _p)

import ml_dtypes

import concourse.bass as bass
import concourse.tile as tile
from concourse import mybir
from concourse.bass_utils import run_bass_kernel_spmd

F32 = mybir.dt.float32
F16 = mybir.dt.float16
FP8 = mybir.dt.float8e4
NP_FP8 = ml_dtypes.float8_e4m3fn
NP_F16 = np.float16
ALU = mybir.AluOpType
AX = mybir.AxisListType
ACTF = mybir.ActivationFunctionType

B, L, D = 64, 197, 768
NCORES = 8
SPC = B // NCORES          # samples per core
T = L - 1                  # 196 text tokens
P = 128                    # compacted patch slots (data max unmasked = 115)
KC = D // 128              # 6 contraction chunks
TXT = KC * T               # 1176 text cols per partition in ti
IMG = KC * P               # 768 image cols
TCHUNKS = ((0, 128), (128, T - 128))   # token partition chunks: 128 + 68

# fp16 const columns: identity | ones | zeros60 ; f32: ones | cls_diff
C16_ID = 0
C16_ONE = 128
C16_Z = 129
C16_W = 193
C32_ONE = 0
C32_CLS = 1
CLS_W = SPC * KC           # 48
C32_W = 1 + CLS_W
NOUT = 2 * SPC + 10        # 16 M-sums + 8 textnorm + gdot + cls


def build_nc(split_waits: bool = True) -> bass.Bass:
    nc = bass.Bass()
    ti = nc.declare_dram_parameter("ti", [SPC, 128, TXT + IMG], FP8, isOutput=False)
    cf16 = nc.declare_dram_parameter("cf16", [128, C16_W], F16, isOutput=False)
    cf32 = nc.declare_dram_parameter("cf32", [128, C32_W], F32, isOutput=False)
    out = nc.declare_dram_parameter("out", [1, NOUT], F32, isOutput=True)

    with tile.TileContext(nc) as tc:
        _emit(nc, tc, ti, cf16, cf32, out)
    if split_waits:  # CoreSim can't execute the injected NoOps; HW needs them
        _split_multiwaits(nc)
    return nc


# The walrus build in this container only supports a single semaphore-wait
# command per instruction. Tile freely attaches several; hoist all but one
# onto same-engine NoOps placed directly before the instruction.
def _split_multiwaits(nc):
    CARRIERS = ("InstNoOp", "InstEventSemaphore")
    for bb in nc.main_func.blocks:
        new = []
        for ins in bb.instructions:
            si = ins.sync_info
            if (
                si is not None
                and si.on_wait
                and len(si.on_wait) > 1
                and type(ins).__name__ not in CARRIERS
            ):
                waits = list(si.on_wait)
                for w in waits[:-1]:
                    nop = mybir.InstNoOp(
                        name=nc.get_next_instruction_name(),
                        engine=ins.engine,
                        ins=[],
                        outs=[],
                        sync_info=mybir.SyncInfo(on_wait=[w], on_update=[]),
                    )
                    new.append(nop)
                ins.sync_info = mybir.SyncInfo(
                    on_wait=[waits[-1]], on_update=list(si.on_update)
                )
            new.append(ins)
        bb.instructions[:] = new


def _emit(nc, tc, ti, cf16, cf32, out):
    with (
        tc.tile_pool(name="data", bufs=1) as data,
        tc.tile_pool(name="work", bufs=1) as work,
        tc.tile_pool(name="psS", bufs=1, space="PSUM") as psS,
        tc.tile_pool(name="psG", bufs=1, space="PSUM") as psG,
        tc.tile_pool(name="psC", bufs=1, space="PSUM") as psC,
        tc.tile_pool(name="psF", bufs=1, space="PSUM") as psF,
    ):
        cf16_sb = work.tile([128, C16_W], F16, tag="cf16")
        nc.sync.dma_start(out=cf16_sb, in_=cf16[:, :])
        cf32_sb = work.tile([128, C32_W], F32, tag="cf32")
        nc.sync.dma_start(out=cf32_sb, in_=cf32[:, :])

        tis = []
        for s in range(SPC):
            t_sb = data.tile([128, TXT + IMG], FP8, tag=f"ti{s}")
            nc.sync.dma_start(out=t_sb, in_=ti[s])
            tis.append(t_sb)

        S_all = psS.tile([128, SPC, 2, P], F32, tag="S")      # 4 banks
        G_all = psG.tile([128, SPC, P], F32, tag="G")         # 2 banks
        cntp = psC.tile([128, SPC], F32, tag="cnt")           # 1 bank
        pf = psF.tile([1, NOUT], F32, tag="pf")               # 1 bank

        Ssb = work.tile([128, SPC, 2, P], F16, tag="Ssb")
        O = work.tile([128, SPC, 2, P], F16, tag="O")
        gm = work.tile([128, SPC, P], F16, tag="gm")
        Mst = work.tile([128, SPC, 2], F16, tag="Mst")
        in2c = work.tile([128, SPC], F32, tag="in2c")
        fst = work.tile([128, 10], F32, tag="fst")
        junk8 = work.tile([128, SPC], F32, tag="junk8")
        cjunk = work.tile([128, CLS_W], F32, tag="cjunk")
        outsb = work.tile([1, NOUT], F32, tag="outsb")

        ones16 = cf16_sb[:, C16_ONE : C16_ONE + 1]
        zeros64 = cf16_sb[0:1, C16_Z : C16_Z + 64]
        anyrow = cf16_sb[0:1, 0:P]
        idn = cf16_sb[:, 0:P]
        ones32 = cf32_sb[:, C32_ONE : C32_ONE + 1]
        clsd = cf32_sb[:, C32_CLS : C32_CLS + CLS_W]

        for g in range(SPC // 2):
            for s in (2 * g, 2 * g + 1):
                tt = tis[s]
                # zero-fill rows 64:128 of the chunk-1 slab (0*x sums); the
                # S group below re-inits rows 64:68 (WAW dep keeps order)
                nc.tensor.matmul(
                    S_all[64:128, s, 1, :], lhsT=zeros64, rhs=anyrow,
                    start=True, stop=True,
                )
                # S[t, p] accumulation over 6 K-chunks, 2 token chunks
                for j, (t0, mj) in enumerate(TCHUNKS):
                    for c in range(KC):
                        nc.tensor.matmul(
                            S_all[0:mj, s, j, :],
                            lhsT=tt[:, c * T + t0 : c * T + t0 + mj],
                            rhs=tt[:, TXT + c * P : TXT + (c + 1) * P],
                            start=(c == 0), stop=(c == KC - 1),
                        )
                # image gram (diag = ||image_p||^2; fp8 squares exact)
                for c in range(KC):
                    it = tt[:, TXT + c * P : TXT + (c + 1) * P]
                    nc.tensor.matmul(
                        G_all[:, s, :], lhsT=it, rhs=it,
                        start=(c == 0), stop=(c == KC - 1),
                    )

            sl = slice(2 * g, 2 * g + 2)
            # f32 PSUM -> fp16 SBUF (ACT engine), then max / one-hot on DVE
            nc.scalar.copy(Ssb[:, sl], S_all[:, sl])
            nc.vector.tensor_reduce(
                out=Mst[:, sl, :], in_=Ssb[:, sl], axis=AX.X, op=ALU.max
            )
            nc.vector.tensor_tensor(
                out=O[:, sl], in0=Ssb[:, sl],
                in1=Mst[:, sl, :].unsqueeze(3).to_broadcast((128, 2, 2, P)),
                op=ALU.is_equal,
            )
            # cnt[p] = sum_t O[t,p]  (patch-on-partition, FD=1)
            for s in (2 * g, 2 * g + 1):
                for j, (t0, mj) in enumerate(TCHUNKS):
                    nc.tensor.matmul(
                        cntp[:, s : s + 1], lhsT=O[0:mj, s, j, :],
                        rhs=ones16[0:mj, :],
                        start=(j == 0), stop=(j == 1),
                    )
            # ||image_p||^2 = diag(G): identity-mask then row-reduce
            nc.vector.tensor_tensor(
                out=gm[:, sl], in0=G_all[:, sl],
                in1=idn.unsqueeze(1).to_broadcast((128, 2, P)),
                op=ALU.mult,
            )
            nc.vector.tensor_reduce(
                out=in2c[:, sl], in_=gm[:, sl], axis=AX.X, op=ALU.add
            )

        # sum_p cnt[p] * ||image_p||^2  (all samples at once; partition partials)
        nc.vector.scalar_tensor_tensor(
            out=junk8, in0=cntp, scalar=1.0, in1=in2c,
            op0=ALU.mult, op1=ALU.mult, accum_out=fst[:, 8:9],
        )
        # CLS loss: host ships (image_cls - target_cls); device squares+sums
        nc.vector.scalar_tensor_tensor(
            out=cjunk, in0=clsd, scalar=1.0, in1=clsd,
            op0=ALU.mult, op1=ALU.mult, accum_out=fst[:, 9:10],
        )
        # ||text||^2: in-place square-accumulate, split ACT/DVE 4+4
        for s in range(SPC):
            txt = tis[s][:, 0:TXT]
            if s < 4:
                nc.scalar.activation(
                    out=txt, in_=txt, func=ACTF.Square,
                    accum_out=fst[:, s : s + 1],
                )
            else:
                nc.vector.scalar_tensor_tensor(
                    out=txt, in0=txt, scalar=1.0, in1=txt,
                    op0=ALU.mult, op1=ALU.mult, accum_out=fst[:, s : s + 1],
                )

        # fold partitions: [1,10] f32 stats | [1,16] fp16 M sums
        nc.tensor.matmul(pf[:, 0:10], lhsT=ones32, rhs=fst, start=True, stop=True)
        nc.tensor.matmul(pf[:, 10:NOUT], lhsT=ones16, rhs=Mst, start=True, stop=True)
        nc.vector.tensor_copy(outsb, pf)
        nc.sync.dma_start(out=out[:, :], in_=outsb)


_NC = None


def _get_nc():
    global _NC
    if _NC is None:
        _NC = build_nc()
    return _NC


def make_in_maps(image, text, target, padding_masks):
    image = np.asarray(image, dtype=np.float32)
    text = np.asarray(text, dtype=np.float32)
    target = np.asarray(target, dtype=np.float32)
    padding_masks = np.asarray(padding_masks)

    idn = np.zeros((128, C16_W), dtype=NP_F16)
    idn[:, 0:P][np.arange(128), np.arange(128)] = 1.0
    idn[:, C16_ONE] = 1.0
    cf16 = idn

    in_maps = []
    for c in range(NCORES):
        sl = slice(c * SPC, (c + 1) * SPC)
        ti = np.zeros((SPC, 128, TXT + IMG), dtype=NP_FP8)
        # text: [s, t, d] -> partition p=d%128, col c*196+t
        tt = text[sl, 1:, :].transpose(0, 2, 1)            # [S, D, T]
        ti[:, :, 0:TXT] = (
            tt.reshape(SPC, KC, 128, T).transpose(0, 2, 1, 3).reshape(SPC, 128, TXT)
            .astype(NP_FP8)
        )
        # image: compact unmasked patches into P slots (zero pad)
        for k, b in enumerate(range(c * SPC, (c + 1) * SPC)):
            keep = np.where(padding_masks[b, 1:] == 0)[0]
            assert len(keep) <= P, f"sample {b}: {len(keep)} unmasked patches > {P}"
            ic = np.zeros((D, P), np.float32)
            ic[:, : len(keep)] = image[b, 1:, :][keep].T
            ti[k, :, TXT:] = (
                ic.reshape(KC, 128, P).transpose(1, 0, 2).reshape(128, IMG)
                .astype(NP_FP8)
            )

        cf32 = np.zeros((128, C32_W), dtype=np.float32)
        cf32[:, C32_ONE] = 1.0
        cf32[:, C32_CLS:] = (
            (image[sl, 0, :] - target[sl, 0, :])
            .reshape(SPC, KC, 128).transpose(2, 0, 1).reshape(128, CLS_W)
        )
        in_maps.append({"ti": ti, "cf16": cf16, "cf32": cf32})
    return in_maps


def combine_outputs(per_core_out):
    tn = 0.0; msum = 0.0; g = 0.0; cls = 0.0
    for r in per_core_out:
        v = np.asarray(r, dtype=np.float64).reshape(NOUT)
        tn += v[0:8].sum()
        g += v[8]
        cls += v[9]
        msum += v[10:].sum()
    kd_token = (tn - 2.0 * msum + g) / (B * T * D)
    kd_cls = cls / (B * D)
    return np.float32(kd_token + kd_cls)


def kernel(image, text, target, padding_masks, _trace=False):
    nc = _get_nc()
    in_maps = make_in_maps(image, text, target, padding_masks)
    res = run_bass_kernel_spmd(nc, in_maps, list(range(NCORES)), trace=_trace)
    loss = combine_outputs([r["out"] for r in res.results])
    if _trace:
        return loss, res
    return loss


# revision 9
# speedup vs baseline: 1.2074x; 1.2074x over previous
"""Trainium2 Bass kernel for nn_CMLITargetLoss (v4: fp8, compaction, lean device).

Reference semantics (B=64, L=197, D=768):
    sim[b,i,t,p] = text[b,t,:] . image[i,p,:]      (masked where padding_masks[b,p])
    token2patch  = argmax over p of sim[:, :, 1:, 1:]
    only the diagonal (b == i) of token2patch is used:
        aligned[b,t] = image[b, 1 + token2patch[b,b,t]]
        kd_token = mean((text[:,1:] - aligned)^2)
    kd_cls  = mean((image[:,0] - target[:,0])^2)
    loss = kd_cls + kd_token

Algebraic reduction (per sample b, tokens t, patches p):
    S[t,p] = text_t . image_p ; M[t] = max_p S ; O = (S == M) one-hot
    sum_t ||text_t - aligned_t||^2
        = sum ||text||^2 - 2*sum M + sum_p cnt[p]*||image_p||^2

Device/host split: inputs ship as fp8e4 (halves HBM traffic; ~3e-4 error).
Per-sample the ~98 unmasked patches are compacted host-side into P=128
slots (zero pad), deleting the mask machinery: zero columns can never win
the argmax (row maxima ~ +70) and pair with ||image_p||^2 = 0 anyway.
The device computes the argmax core -- all S matmuls, row maxima, the
equality one-hot, patch counts, the cnt.in2 dot, and the CLS MSE -- plus
the partition folds. The two input self-statistics (per-sample
sum||text||^2, per-patch ||image_p||^2) are computed on host from the
SAME fp8 values during packing; measured v2/v3 traces show they cannot
fit beside max/one-hot on the two usable elementwise engines (GpSimd has
no ALU ops in this toolchain and ACT/DVE are saturated).

Measured-trace-driven details: all 8 samples' S live in ONE 4-bank PSUM
tile [128,8,2,128] so max/one-hot run as 2-sample slabs (DVE/ACT pay
58-352 cycles fixed per instruction); ~3.4us of dummy FD=512 matmuls
run during the DMA wait so the PE HAM clock-gate opens (1.2->2.4 GHz)
before real work; DMA triggers cost ~730ns each on a hwdge engine, so
they are split across Sync and ACT; rows 64:128 of each chunk-1 slab
are zero-filled by a K=1 matmul so reduces never see stale PSUM.
"""

import os
import sys

import numpy as np

for _p in ("/opt/trn_rl_repo", "/root/.axon_site/_ro/trn_rl_repo"):
    if _p not in sys.path and os.path.isdir(_p):
        sys.path.insert(0, _p)

import ml_dtypes

import concourse.bass as bass
import concourse.tile as tile
from concourse import mybir
from concourse.bass_utils import run_bass_kernel_spmd

F32 = mybir.dt.float32
F16 = mybir.dt.float16
FP8 = mybir.dt.float8e4
NP_FP8 = ml_dtypes.float8_e4m3fn
NP_F16 = np.float16
ALU = mybir.AluOpType
AX = mybir.AxisListType

B, L, D = 64, 197, 768
NCORES = 8
SPC = B // NCORES          # samples per core
T = L - 1                  # 196 text tokens
P = 128                    # compacted patch slots (data max unmasked = 115)
KC = D // 128              # 6 contraction chunks
TXT = KC * T               # 1176 text cols per partition in ti
IMG = KC * P               # 768 image cols
TCHUNKS = ((0, 128), (128, T - 128))   # token partition chunks: 128 + 68

# fp16 consts: ones | zeros64 ; f32 consts: ones | cls_diff | in2
C16_ONE = 0
C16_Z = 1
C16_W = 129
C32_ONE = 0
C32_CLS = 1
CLS_W = SPC * KC           # 48
C32_IN2 = C32_CLS + CLS_W  # in2[p, s] = ||image_p||^2, [128, 8]
C32_W = C32_IN2 + SPC
NOUT = 2 * SPC + 2         # 16 M-sums + gdot + cls


def build_nc(split_waits: bool = True) -> bass.Bass:
    nc = bass.Bass()
    ti = nc.declare_dram_parameter("ti", [SPC, 128, TXT + IMG], FP8, isOutput=False)
    cf16 = nc.declare_dram_parameter("cf16", [128, C16_W], F16, isOutput=False)
    cf32 = nc.declare_dram_parameter("cf32", [128, C32_W], F32, isOutput=False)
    out = nc.declare_dram_parameter("out", [1, NOUT], F32, isOutput=True)

    with tile.TileContext(nc) as tc:
        _emit(nc, tc, ti, cf16, cf32, out)
    if split_waits:  # CoreSim can't execute the injected NoOps; HW needs them
        _split_multiwaits(nc)
    return nc


# The walrus build in this container only supports a single semaphore-wait
# command per instruction. Tile freely attaches several; hoist all but one
# onto same-engine NoOps placed directly before the instruction.
def _split_multiwaits(nc):
    CARRIERS = ("InstNoOp", "InstEventSemaphore")
    for bb in nc.main_func.blocks:
        new = []
        for ins in bb.instructions:
            si = ins.sync_info
            if (
                si is not None
                and si.on_wait
                and len(si.on_wait) > 1
                and type(ins).__name__ not in CARRIERS
            ):
                waits = list(si.on_wait)
                for w in waits[:-1]:
                    nop = mybir.InstNoOp(
                        name=nc.get_next_instruction_name(),
                        engine=ins.engine,
                        ins=[],
                        outs=[],
                        sync_info=mybir.SyncInfo(on_wait=[w], on_update=[]),
                    )
                    new.append(nop)
                ins.sync_info = mybir.SyncInfo(
                    on_wait=[waits[-1]], on_update=list(si.on_update)
                )
            new.append(ins)
        bb.instructions[:] = new


def _emit(nc, tc, ti, cf16, cf32, out):
    with (
        tc.tile_pool(name="data", bufs=1) as data,
        tc.tile_pool(name="work", bufs=1) as work,
        tc.tile_pool(name="psS", bufs=1, space="PSUM") as psS,
        tc.tile_pool(name="psC", bufs=1, space="PSUM") as psC,
        tc.tile_pool(name="psF", bufs=1, space="PSUM") as psF,
    ):
        cf16_sb = work.tile([128, C16_W], F16, tag="cf16")
        nc.sync.dma_start(out=cf16_sb, in_=cf16[:, :])
        cf32_sb = work.tile([128, C32_W], F32, tag="cf32")
        nc.sync.dma_start(out=cf32_sb, in_=cf32[:, :])

        # ti DMA triggers split across the two hwdge engines (Sync, ACT):
        # each DMA_DIRECT2D costs ~730ns of trigger-issue on its engine.
        tis = []
        for s in range(SPC):
            t_sb = data.tile([128, TXT + IMG], FP8, tag=f"ti{s}")
            eng = nc.sync if s < 2 else nc.scalar
            eng.dma_start(out=t_sb, in_=ti[s])
            tis.append(t_sb)

        S_all = psS.tile([128, SPC, 2, P], F32, tag="S")      # 4 banks
        cntp = psC.tile([128, SPC], F32, tag="cnt")           # 1 bank
        pf = psF.tile([1, 512], F32, tag="pf")                # 1 bank; [0:NOUT] real

        # HAM warm-up: ~3.4us of dummy PE work during the DMA wait so the
        # clock gate opens (1.2 -> 2.4 GHz) before the real matmuls start.
        dummy = work.tile([1, 512], F16, tag="dummy")
        nc.gpsimd.memset(dummy, 0.0)
        for _ in range(8):
            nc.tensor.matmul(
                pf[0:1, :], lhsT=dummy[0:1, 0:1], rhs=dummy[0:1, :],
                start=True, stop=True,
            )

        Ssb = work.tile([128, SPC, 2, P], F16, tag="Ssb")
        O = work.tile([128, SPC, 2, P], F16, tag="O")
        Mst = work.tile([128, SPC, 2], F16, tag="Mst")
        fst = work.tile([128, 2], F32, tag="fst")
        junk8 = work.tile([128, SPC], F32, tag="junk8")
        cjunk = work.tile([128, CLS_W], F32, tag="cjunk")
        outsb = work.tile([1, NOUT], F32, tag="outsb")

        ones16 = cf16_sb[:, C16_ONE : C16_ONE + 1]
        zeros64 = cf16_sb[0:1, C16_Z : C16_Z + 64]
        anyrow = cf16_sb[0:1, 0:P]
        ones32 = cf32_sb[:, C32_ONE : C32_ONE + 1]
        clsd = cf32_sb[:, C32_CLS : C32_CLS + CLS_W]
        in2c = cf32_sb[:, C32_IN2 : C32_IN2 + SPC]

        for g in range(SPC // 2):
            for s in (2 * g, 2 * g + 1):
                tt = tis[s]
                # zero-fill rows 64:128 of the chunk-1 slab (0*x sums); the
                # S group below re-inits rows 64:68 (WAW dep keeps order)
                nc.tensor.matmul(
                    S_all[64:128, s, 1, :], lhsT=zeros64, rhs=anyrow,
                    start=True, stop=True,
                )
                # S[t, p] accumulation over 6 K-chunks, 2 token chunks
                for j, (t0, mj) in enumerate(TCHUNKS):
                    for c in range(KC):
                        nc.tensor.matmul(
                            S_all[0:mj, s, j, :],
                            lhsT=tt[:, c * T + t0 : c * T + t0 + mj],
                            rhs=tt[:, TXT + c * P : TXT + (c + 1) * P],
                            start=(c == 0), stop=(c == KC - 1),
                        )

            sl = slice(2 * g, 2 * g + 2)
            # f32 PSUM -> fp16 SBUF (ACT engine), then max / one-hot on DVE
            nc.scalar.copy(Ssb[:, sl], S_all[:, sl])
            nc.vector.tensor_reduce(
                out=Mst[:, sl, :], in_=Ssb[:, sl], axis=AX.X, op=ALU.max
            )
            nc.vector.tensor_tensor(
                out=O[:, sl], in0=Ssb[:, sl],
                in1=Mst[:, sl, :].unsqueeze(3).to_broadcast((128, 2, 2, P)),
                op=ALU.is_equal,
            )
            # cnt[p] = sum_t O[t,p]  (patch-on-partition, FD=1)
            for s in (2 * g, 2 * g + 1):
                for j, (t0, mj) in enumerate(TCHUNKS):
                    nc.tensor.matmul(
                        cntp[:, s : s + 1], lhsT=O[0:mj, s, j, :],
                        rhs=ones16[0:mj, :],
                        start=(j == 0), stop=(j == 1),
                    )

        # sum_p cnt[p] * ||image_p||^2  (all samples; partition partials)
        nc.vector.scalar_tensor_tensor(
            out=junk8, in0=cntp, scalar=1.0, in1=in2c,
            op0=ALU.mult, op1=ALU.mult, accum_out=fst[:, 0:1],
        )
        # CLS loss: host ships (image_cls - target_cls); device squares+sums
        nc.vector.scalar_tensor_tensor(
            out=cjunk, in0=clsd, scalar=1.0, in1=clsd,
            op0=ALU.mult, op1=ALU.mult, accum_out=fst[:, 1:2],
        )

        # fold partitions: [1,2] f32 stats | [1,16] fp16 M sums
        nc.tensor.matmul(pf[:, 0:2], lhsT=ones32, rhs=fst, start=True, stop=True)
        nc.tensor.matmul(pf[:, 2:NOUT], lhsT=ones16, rhs=Mst, start=True, stop=True)
        nc.vector.tensor_copy(outsb, pf[:, 0:NOUT])
        nc.sync.dma_start(out=out[:, :], in_=outsb)


_NC = None


def _get_nc():
    global _NC
    if _NC is None:
        _NC = build_nc()
    return _NC


def make_in_maps(image, text, target, padding_masks):
    image = np.asarray(image, dtype=np.float32)
    text = np.asarray(text, dtype=np.float32)
    target = np.asarray(target, dtype=np.float32)
    padding_masks = np.asarray(padding_masks)

    cf16 = np.zeros((128, C16_W), dtype=NP_F16)
    cf16[:, C16_ONE] = 1.0

    in_maps = []
    textnorms = []
    for c in range(NCORES):
        sl = slice(c * SPC, (c + 1) * SPC)
        ti = np.zeros((SPC, 128, TXT + IMG), dtype=NP_FP8)
        # text: [s, t, d] -> partition p = d % 128, col c*196 + t
        tt = text[sl, 1:, :].transpose(0, 2, 1)            # [S, D, T]
        ti[:, :, 0:TXT] = (
            tt.reshape(SPC, KC, 128, T).transpose(0, 2, 1, 3).reshape(SPC, 128, TXT)
            .astype(NP_FP8)
        )
        # image: compact unmasked patches into P slots (zero pad)
        in2 = np.zeros((128, SPC), dtype=np.float32)
        for k, b in enumerate(range(c * SPC, (c + 1) * SPC)):
            keep = np.where(padding_masks[b, 1:] == 0)[0]
            assert len(keep) <= P, f"sample {b}: {len(keep)} unmasked patches > {P}"
            ic = np.zeros((D, P), np.float32)
            ic[:, : len(keep)] = image[b, 1:, :][keep].T
            icq = ic.astype(NP_FP8)
            ti[k, :, TXT:] = (
                icq.reshape(KC, 128, P).transpose(1, 0, 2).reshape(128, IMG)
            )
            # ||image_p||^2 from the SAME fp8 values the device multiplies
            in2[:, k] = (icq.astype(np.float32) ** 2).sum(axis=0)
        # per-sample sum||text||^2 from the shipped fp8 values
        tn = (ti[:, :, 0:TXT].astype(np.float64) ** 2).sum(axis=(1, 2))
        textnorms.append(tn)

        cf32 = np.zeros((128, C32_W), dtype=np.float32)
        cf32[:, C32_ONE] = 1.0
        cf32[:, C32_CLS : C32_CLS + CLS_W] = (
            (image[sl, 0, :] - target[sl, 0, :])
            .reshape(SPC, KC, 128).transpose(2, 0, 1).reshape(128, CLS_W)
        )
        cf32[:, C32_IN2 : C32_IN2 + SPC] = in2
        in_maps.append({"ti": ti, "cf16": cf16, "cf32": cf32})
    return in_maps, textnorms


def combine_outputs(per_core_out, textnorms):
    tn = float(np.sum([t.sum() for t in textnorms]))
    msum = 0.0; g = 0.0; cls = 0.0
    for r in per_core_out:
        v = np.asarray(r, dtype=np.float64).reshape(NOUT)
        g += v[0]
        cls += v[1]
        msum += v[2:].sum()
    kd_token = (tn - 2.0 * msum + g) / (B * T * D)
    kd_cls = cls / (B * D)
    return np.float32(kd_token + kd_cls)


def kernel(image, text, target, padding_masks, _trace=False):
    nc = _get_nc()
    in_maps, textnorms = make_in_maps(image, text, target, padding_masks)
    res = run_bass_kernel_spmd(nc, in_maps, list(range(NCORES)), trace=_trace)
    loss = combine_outputs([r["out"] for r in res.results], textnorms)
    if _trace:
        return loss, res
    return loss


# revision 10
# speedup vs baseline: 1.2468x; 1.0326x over previous
"""Trainium2 Bass kernel for nn_CMLITargetLoss (v4: fp8, compaction, lean device).

Reference semantics (B=64, L=197, D=768):
    sim[b,i,t,p] = text[b,t,:] . image[i,p,:]      (masked where padding_masks[b,p])
    token2patch  = argmax over p of sim[:, :, 1:, 1:]
    only the diagonal (b == i) of token2patch is used:
        aligned[b,t] = image[b, 1 + token2patch[b,b,t]]
        kd_token = mean((text[:,1:] - aligned)^2)
    kd_cls  = mean((image[:,0] - target[:,0])^2)
    loss = kd_cls + kd_token

Algebraic reduction (per sample b, tokens t, patches p):
    S[t,p] = text_t . image_p ; M[t] = max_p S ; O = (S == M) one-hot
    sum_t ||text_t - aligned_t||^2
        = sum ||text||^2 - 2*sum M + sum_p cnt[p]*||image_p||^2

Device/host split: inputs ship as fp8e4 (halves HBM traffic; ~3e-4 error).
Per-sample the ~98 unmasked patches are compacted host-side into P=128
slots (zero pad), deleting the mask machinery: zero columns can never win
the argmax (row maxima ~ +70) and pair with ||image_p||^2 = 0 anyway.
The device computes the argmax core -- all S matmuls, row maxima, the
equality one-hot, patch counts, the cnt.in2 dot, and the CLS MSE -- plus
the partition folds. The two input self-statistics (per-sample
sum||text||^2, per-patch ||image_p||^2) are computed on host from the
SAME fp8 values during packing; measured v2/v3 traces show they cannot
fit beside max/one-hot on the two usable elementwise engines (GpSimd has
no ALU ops in this toolchain and ACT/DVE are saturated).

Measured-trace-driven details: all 8 samples' S live in ONE 4-bank PSUM
tile [128,8,2,128] so max/one-hot run as 2-sample slabs (DVE/ACT pay
58-352 cycles fixed per instruction); ~3.4us of dummy FD=512 matmuls
run during the DMA wait so the PE HAM clock-gate opens (1.2->2.4 GHz)
before real work; DMA triggers cost ~730ns each on a hwdge engine, so
they are split across Sync and ACT; rows 64:128 of each chunk-1 slab
are zero-filled by a K=1 matmul so reduces never see stale PSUM.
"""

import os
import sys

import numpy as np

for _p in ("/opt/trn_rl_repo", "/root/.axon_site/_ro/trn_rl_repo"):
    if _p not in sys.path and os.path.isdir(_p):
        sys.path.insert(0, _p)

import ml_dtypes

import concourse.bass as bass
import concourse.tile as tile
from concourse import mybir
from concourse.bass_utils import run_bass_kernel_spmd

F32 = mybir.dt.float32
F16 = mybir.dt.float16
FP8 = mybir.dt.float8e4
NP_FP8 = ml_dtypes.float8_e4m3fn
NP_F16 = np.float16
ALU = mybir.AluOpType
AX = mybir.AxisListType

B, L, D = 64, 197, 768
NCORES = 8
SPC = B // NCORES          # samples per core
T = L - 1                  # 196 text tokens
P = 128                    # compacted patch slots (data max unmasked = 115)
KC = D // 128              # 6 contraction chunks
TXT = KC * T               # 1176 text cols per partition in ti
IMG = KC * P               # 768 image cols
TCHUNKS = ((0, 128), (128, T - 128))   # token partition chunks: 128 + 68

# fp16 consts: ones | zeros64 ; f32 consts: ones | cls_diff | in2
C16_ONE = 0
C16_Z = 1
C16_W = 129
C32_ONE = 0
C32_CLS = 1
CLS_W = SPC * KC           # 48
C32_IN2 = C32_CLS + CLS_W  # in2[p, s] = ||image_p||^2, [128, 8]
C32_W = C32_IN2 + SPC
NOUT = 2 * SPC + 2         # 16 M-sums + gdot + cls


def build_nc(split_waits: bool = True) -> bass.Bass:
    nc = bass.Bass()
    ti = nc.declare_dram_parameter("ti", [KC, 128, SPC, T + P], FP8, isOutput=False)
    cf16 = nc.declare_dram_parameter("cf16", [128, C16_W], F16, isOutput=False)
    cf32 = nc.declare_dram_parameter("cf32", [128, C32_W], F32, isOutput=False)
    out = nc.declare_dram_parameter("out", [1, NOUT], F32, isOutput=True)

    with tile.TileContext(nc) as tc:
        _emit(nc, tc, ti, cf16, cf32, out)
    if split_waits:  # CoreSim can't execute the injected NoOps; HW needs them
        _split_multiwaits(nc)
    return nc


# The walrus build in this container only supports a single semaphore-wait
# command per instruction. Tile freely attaches several; hoist all but one
# onto same-engine NoOps placed directly before the instruction.
def _split_multiwaits(nc):
    CARRIERS = ("InstNoOp", "InstEventSemaphore")
    for bb in nc.main_func.blocks:
        new = []
        for ins in bb.instructions:
            si = ins.sync_info
            if (
                si is not None
                and si.on_wait
                and len(si.on_wait) > 1
                and type(ins).__name__ not in CARRIERS
            ):
                waits = list(si.on_wait)
                for w in waits[:-1]:
                    nop = mybir.InstNoOp(
                        name=nc.get_next_instruction_name(),
                        engine=ins.engine,
                        ins=[],
                        outs=[],
                        sync_info=mybir.SyncInfo(on_wait=[w], on_update=[]),
                    )
                    new.append(nop)
                ins.sync_info = mybir.SyncInfo(
                    on_wait=[waits[-1]], on_update=list(si.on_update)
                )
            new.append(ins)
        bb.instructions[:] = new


def _emit(nc, tc, ti, cf16, cf32, out):
    with (
        tc.tile_pool(name="data", bufs=1) as data,
        tc.tile_pool(name="work", bufs=1) as work,
        tc.tile_pool(name="psS", bufs=1, space="PSUM") as psS,
        tc.tile_pool(name="psC", bufs=1, space="PSUM") as psC,
        tc.tile_pool(name="psF", bufs=1, space="PSUM") as psF,
    ):
        cf16_sb = work.tile([128, C16_W], F16, tag="cf16")
        nc.sync.dma_start(out=cf16_sb, in_=cf16[:, :])
        cf32_sb = work.tile([128, C32_W], F32, tag="cf32")
        nc.sync.dma_start(out=cf32_sb, in_=cf32[:, :])

        # chunk-major input: 6 DMAs, each one K-chunk (text+image, all
        # samples, 2.6KB/partition) so S accumulation starts on chunk 0
        # while later chunks stream. Triggers (~730ns each) alternate
        # between the two hwdge engines (Sync, ACT).
        tcs = []
        for c in range(KC):
            t_sb = data.tile([128, SPC, T + P], FP8, tag=f"tc{c}")
            eng = nc.sync if c % 2 == 0 else nc.scalar
            eng.dma_start(out=t_sb, in_=ti[c])
            tcs.append(t_sb)

        S_all = psS.tile([128, SPC, 2, P], F32, tag="S")      # 4 banks
        cntp = psC.tile([128, SPC], F32, tag="cnt")           # 1 bank
        pf = psF.tile([1, 512], F32, tag="pf")                # 1 bank; [0:NOUT] real

        # HAM warm-up: ~3.4us of dummy PE work during the DMA wait so the
        # clock gate opens (1.2 -> 2.4 GHz) before the real matmuls start.
        dummy = work.tile([1, 512], F16, tag="dummy")
        nc.gpsimd.memset(dummy, 0.0)
        for _ in range(8):
            nc.tensor.matmul(
                pf[0:1, :], lhsT=dummy[0:1, 0:1], rhs=dummy[0:1, :],
                start=True, stop=True,
            )

        Ssb = work.tile([128, SPC, 2, P], F16, tag="Ssb")
        O = work.tile([128, SPC, 2, P], F16, tag="O")
        Mst = work.tile([128, SPC, 2], F16, tag="Mst")
        fst = work.tile([128, 2], F32, tag="fst")
        junk8 = work.tile([128, SPC], F32, tag="junk8")
        cjunk = work.tile([128, CLS_W], F32, tag="cjunk")
        outsb = work.tile([1, NOUT], F32, tag="outsb")

        ones16 = cf16_sb[:, C16_ONE : C16_ONE + 1]
        zeros64 = cf16_sb[0:1, C16_Z : C16_Z + 64]
        anyrow = cf16_sb[0:1, 0:P]
        ones32 = cf32_sb[:, C32_ONE : C32_ONE + 1]
        clsd = cf32_sb[:, C32_CLS : C32_CLS + CLS_W]
        in2c = cf32_sb[:, C32_IN2 : C32_IN2 + SPC]

        # zero-fill rows 64:128 of each chunk-1 slab (0*x sums); the S
        # groups re-init rows 64:68 (WAW dep keeps order). No data dep ->
        # these also run early and keep the PE HAM window busy.
        for s in range(SPC):
            nc.tensor.matmul(
                S_all[64:128, s, 1, :], lhsT=zeros64, rhs=anyrow,
                start=True, stop=True,
            )
        # S accumulation, chunk-major: all (sample, token-chunk) groups
        # advance as each chunk's DMA lands
        for c in range(KC):
            for s in range(SPC):
                for j, (t0, mj) in enumerate(TCHUNKS):
                    nc.tensor.matmul(
                        S_all[0:mj, s, j, :],
                        lhsT=tcs[c][:, s, t0 : t0 + mj],
                        rhs=tcs[c][:, s, T : T + P],
                        start=(c == 0), stop=(c == KC - 1),
                    )

        for g in range(SPC // 2):
            sl = slice(2 * g, 2 * g + 2)
            # f32 PSUM -> fp16 SBUF (ACT engine), then max / one-hot on DVE
            nc.scalar.copy(Ssb[:, sl], S_all[:, sl])
            nc.vector.tensor_reduce(
                out=Mst[:, sl, :], in_=Ssb[:, sl], axis=AX.X, op=ALU.max
            )
            nc.vector.tensor_tensor(
                out=O[:, sl], in0=Ssb[:, sl],
                in1=Mst[:, sl, :].unsqueeze(3).to_broadcast((128, 2, 2, P)),
                op=ALU.is_equal,
            )
            # cnt[p] = sum_t O[t,p]  (patch-on-partition, FD=1)
            for s in (2 * g, 2 * g + 1):
                for j, (t0, mj) in enumerate(TCHUNKS):
                    nc.tensor.matmul(
                        cntp[:, s : s + 1], lhsT=O[0:mj, s, j, :],
                        rhs=ones16[0:mj, :],
                        start=(j == 0), stop=(j == 1),
                    )

        # sum_p cnt[p] * ||image_p||^2  (all samples; partition partials)
        nc.vector.scalar_tensor_tensor(
            out=junk8, in0=cntp, scalar=1.0, in1=in2c,
            op0=ALU.mult, op1=ALU.mult, accum_out=fst[:, 0:1],
        )
        # CLS loss: host ships (image_cls - target_cls); device squares+sums
        nc.vector.scalar_tensor_tensor(
            out=cjunk, in0=clsd, scalar=1.0, in1=clsd,
            op0=ALU.mult, op1=ALU.mult, accum_out=fst[:, 1:2],
        )

        # fold partitions: [1,2] f32 stats | [1,16] fp16 M sums
        nc.tensor.matmul(pf[:, 0:2], lhsT=ones32, rhs=fst, start=True, stop=True)
        nc.tensor.matmul(pf[:, 2:NOUT], lhsT=ones16, rhs=Mst, start=True, stop=True)
        nc.vector.tensor_copy(outsb, pf[:, 0:NOUT])
        nc.sync.dma_start(out=out[:, :], in_=outsb)


_NC = None


def _get_nc():
    global _NC
    if _NC is None:
        _NC = build_nc()
    return _NC


def make_in_maps(image, text, target, padding_masks):
    image = np.asarray(image, dtype=np.float32)
    text = np.asarray(text, dtype=np.float32)
    target = np.asarray(target, dtype=np.float32)
    padding_masks = np.asarray(padding_masks)

    cf16 = np.zeros((128, C16_W), dtype=NP_F16)
    cf16[:, C16_ONE] = 1.0

    in_maps = []
    textnorms = []
    for c in range(NCORES):
        sl = slice(c * SPC, (c + 1) * SPC)
        ti = np.zeros((KC, 128, SPC, T + P), dtype=NP_FP8)
        # text: [s, t, d] -> chunk c = d//128, partition p = d%128
        ttq = text[sl, 1:, :].transpose(0, 2, 1).astype(NP_FP8)   # [S, D, T]
        ti[:, :, :, 0:T] = ttq.reshape(SPC, KC, 128, T).transpose(1, 2, 0, 3)
        # image: compact unmasked patches into P slots (zero pad)
        in2 = np.zeros((128, SPC), dtype=np.float32)
        for k, b in enumerate(range(c * SPC, (c + 1) * SPC)):
            keep = np.where(padding_masks[b, 1:] == 0)[0]
            assert len(keep) <= P, f"sample {b}: {len(keep)} unmasked patches > {P}"
            ic = np.zeros((D, P), np.float32)
            ic[:, : len(keep)] = image[b, 1:, :][keep].T
            icq = ic.astype(NP_FP8)
            ti[:, :, k, T:] = icq.reshape(KC, 128, P)
            # ||image_p||^2 from the SAME fp8 values the device multiplies
            in2[:, k] = (icq.astype(np.float32) ** 2).sum(axis=0)
        # per-sample sum||text||^2 from the shipped fp8 values
        tn = (ttq.astype(np.float64) ** 2).sum(axis=(1, 2))
        textnorms.append(tn)

        cf32 = np.zeros((128, C32_W), dtype=np.float32)
        cf32[:, C32_ONE] = 1.0
        cf32[:, C32_CLS : C32_CLS + CLS_W] = (
            (image[sl, 0, :] - target[sl, 0, :])
            .reshape(SPC, KC, 128).transpose(2, 0, 1).reshape(128, CLS_W)
        )
        cf32[:, C32_IN2 : C32_IN2 + SPC] = in2
        in_maps.append({"ti": ti, "cf16": cf16, "cf32": cf32})
    return in_maps, textnorms


def combine_outputs(per_core_out, textnorms):
    tn = float(np.sum([t.sum() for t in textnorms]))
    msum = 0.0; g = 0.0; cls = 0.0
    for r in per_core_out:
        v = np.asarray(r, dtype=np.float64).reshape(NOUT)
        g += v[0]
        cls += v[1]
        msum += v[2:].sum()
    kd_token = (tn - 2.0 * msum + g) / (B * T * D)
    kd_cls = cls / (B * D)
    return np.float32(kd_token + kd_cls)


def kernel(image, text, target, padding_masks, _trace=False):
    nc = _get_nc()
    in_maps, textnorms = make_in_maps(image, text, target, padding_masks)
    res = run_bass_kernel_spmd(nc, in_maps, list(range(NCORES)), trace=_trace)
    loss = combine_outputs([r["out"] for r in res.results], textnorms)
    if _trace:
        return loss, res
    return loss
